# revision 73
# baseline (speedup 1.0000x reference)
"""Trainium2 Bass kernel for nn_BlockwiseAttention (sparse_attention).

Full (unsharded) inputs in, full output out.  Internally shards across the
8 NeuronCores as (batch x head-group): core c -> batch c//4, heads
[4*(c%4), 4*(c%4)+4).  Each core computes qkv projection + RoPE + masked
attention + its slice of the output projection; the host sums the per-core
(and per head-pair) partial projections.

v2 design (ACT-bound softmax pipeline):
  - all matmuls in bf16 (cost-model rate 1.0/row, same as fp32r, but
    enables small-free-dim AV matmuls without the fp32r <256 penalty)
  - scores^T tiles [t=128, s=1024] in PSUM; exp on ACT -> unn bf16 SBUF
  - mask applied POST-exp as zeroing: DVE triangle multiplies + Pool
    memset on the in-band blocks (cheap; frees the PE of mask matmuls)
  - AV restructured: out[s,d] (128-partition outputs, 64-wide free) --
    half the PE rows of the [d,s] orientation; denominators via [128,1]
    ones-matmuls into a separate PSUM bank
  - normalize (DVE tensor_scalar), PE-transpose o -> oT [d,s], out_proj
    per head-pair with partial sums finished on the host
  - q23/k23 projections, v projections, and pair-0 out_proj are
    interleaved into the attention T-loops (PE has slack; ACT is the
    bottleneck engine at ~133us busy)
"""

import sys
import numpy as np
from collections import deque

for _p in ("/opt/trn_rl_repo",):
    if _p not in sys.path:
        sys.path.insert(0, _p)

import ml_dtypes

import concourse.bass as bass
import concourse.tile as tile
from concourse import mybir
from concourse.bass import ts, ds
from concourse.bass_utils import run_bass_kernel_spmd

# ---------------------------------------------------------------- constants
B, S, E = 2, 2048, 1024
H, D = 16, 64
HL = 4                     # heads per core
N_CORES = 8
BLOCK, NG, WIN = 256, 4, 128
ROPE_BASE = 10000.0

F32 = mybir.dt.float32
BF16 = mybir.dt.bfloat16
EXP = mybir.ActivationFunctionType.Exp

NT = S // 128              # 16 t-chunks
_SHUF = [i ^ 1 for i in range(32)]


# ---------------------------------------------------------------- program
def build_nc(for_sim: bool = False) -> bass.Bass:
    nc = bass.Bass()

    xt = nc.declare_dram_parameter("xt", [E, S], BF16, isOutput=False)
    wqkv = nc.declare_dram_parameter("wqkv", [E, 768], BF16, isOutput=False)
    wout = nc.declare_dram_parameter("wout", [256, E], BF16, isOutput=False)
    cosb = nc.declare_dram_parameter("cosb", [128, S], BF16, isOutput=False)
    sinb = nc.declare_dram_parameter("sinb", [128, S], BF16, isOutput=False)
    trid = nc.declare_dram_parameter("trid", [128, 256], BF16, isOutput=False)
    t0row = nc.declare_dram_parameter("t0row", [4, S], BF16, isOutput=False)
    ident = nc.declare_dram_parameter("ident", [128, 128], BF16,
                                      isOutput=False)
    yt2 = nc.declare_dram_parameter("yt2", [2, E, S], BF16, isOutput=True)

    with tile.TileContext(nc) as tc:
        _body(nc, tc, xt, wqkv, wout, cosb, sinb, trid, t0row, ident, yt2)
    if not for_sim:
        _split_waits(nc)
    return nc


def _split_waits(nc):
    """walrus's per-instruction sync structs accept few sync waits; peel
    extra waits onto same-engine ENGINE_NOPs inserted right before the
    instruction (the sequencer processes their waits first)."""
    eng = mybir.EngineType
    builders = {eng.PE: nc.tensor, eng.DVE: nc.vector,
                eng.Activation: nc.scalar, eng.Pool: nc.gpsimd,
                eng.SP: nc.sync}
    fn = nc.m.functions[0]

    def mk_nop(builder):
        builder.nop()
        scratch = fn.blocks[-1].instructions
        raw = scratch.pop()
        assert type(raw).__name__ == "InstNoOp", type(raw).__name__
        return raw

    for blk in fn.blocks:
        out = []
        changed = False
        for inst in blk.instructions:
            si = inst.sync_info
            if (si is not None and len(si.on_wait) > 1
                    and inst.engine in builders):
                waits = list(si.on_wait)
                for w in waits[:-1]:
                    nop = mk_nop(builders[inst.engine])
                    nop.engine = inst.engine
                    nop.sync_info = mybir.SyncInfo(on_wait=[w], on_update=[])
                    out.append(nop)
                si.on_wait = [waits[-1]]
                changed = True
            out.append(inst)
        if changed:
            blk.instructions[:] = out
    return nc


def _body(nc, tc, xt, wqkv, wout, cosb, sinb, trid, t0row, ident, yt2):
    from contextlib import ExitStack

    with ExitStack() as ctx:
        # ---------------- SBUF pools -------------------------------------
        consts = ctx.enter_context(tc.tile_pool(name="consts", bufs=1))
        xt_p = ctx.enter_context(tc.tile_pool(name="xt_p", bufs=1))
        w_p = ctx.enter_context(tc.tile_pool(name="w_p", bufs=1))
        qk_sb = ctx.enter_context(tc.tile_pool(name="qk_sb", bufs=1))
        v_sb = ctx.enter_context(tc.tile_pool(name="v_sb", bufs=1))
        rope_p = ctx.enter_context(tc.tile_pool(name="rope_p", bufs=2))
        unn_p = ctx.enter_context(tc.tile_pool(name="unn_p", bufs=8))
        osb_p = ctx.enter_context(tc.tile_pool(name="osb_p", bufs=4))
        rec_p = ctx.enter_context(tc.tile_pool(name="rec_p", bufs=2))
        oT_p = ctx.enter_context(tc.tile_pool(name="oT_p", bufs=1))
        ysb_p = ctx.enter_context(tc.tile_pool(name="ysb_p", bufs=10))

        # ---------------- PSUM pools (16KB budget, creation order) -------
        qk_ps = ctx.enter_context(
            tc.tile_pool(name="qk_ps", bufs=2, space="PSUM"))   # 8KB
        av_ps = ctx.enter_context(
            tc.tile_pool(name="av_ps", bufs=1, space="PSUM"))   # 2KB
        dn_ps = ctx.enter_context(
            tc.tile_pool(name="dn_ps", bufs=1, space="PSUM"))   # 2KB
        y_ps = ctx.enter_context(
            tc.tile_pool(name="y_ps", bufs=1, space="PSUM"))    # 2KB
        tr_ps = ctx.enter_context(
            tc.tile_pool(name="tr_ps", bufs=1, space="PSUM"))   # 1 bank

        # ---------------- SBUF tiles -------------------------------------
        cos_t = consts.tile([128, S], BF16, tag="cos")
        sin_t = consts.tile([128, S], BF16, tag="sin")
        tri_t = consts.tile([128, 256], BF16, tag="tri")
        t0_t = consts.tile([4, S], BF16, tag="t0")
        ident_t = consts.tile([128, 128], BF16, tag="ident")
        wout_t = [consts.tile([128, E], BF16, tag=f"wout{p}",
                              name=f"wout{p}") for p in range(2)]
        ones1 = consts.tile([128, 1], BF16, tag="ones1")

        xt_t = [xt_p.tile([128, S], BF16, tag=f"xt{k}", name=f"xt{k}")
                for k in range(8)]
        # wb[m]: m0=q01 m1=q23 m2=k01 m3=k23 ([128, 8 e-chunks x 128 cols])
        wb = [w_p.tile([128, 1024], BF16, tag=f"wb{m}", name=f"wb{m}")
              for m in range(4)]
        wv = w_p.tile([128, 2048], BF16, tag="wv")
        qkT = [qk_sb.tile([128, S], BF16, tag=f"qkT{m}", name=f"qkT{m}")
               for m in range(4)]
        # v_all block (T, h) at col 256*T + 64*h
        v_all = v_sb.tile([128, NT * 256], BF16, tag="vall")
        oT2 = [oT_p.tile([128, S], BF16, tag=f"oT{p}", name=f"oT{p}")
               for p in range(2)]

        # ---------------- DMA queues -------------------------------------
        # HWDGE(sync) takes the bulk transfers: SWDGE descriptor generation
        # occupies the Pool ENGINE (~2us per big transfer), which we need
        # for compute copies.  gpsimd only loads the small weight blocks.
        nc.gpsimd.dma_start(
            out=wb[0][:].rearrange("p (k c) -> p k c", k=8),
            in_=wqkv[:, 0:128].rearrange("(k p) c -> p k c", p=128))
        nc.scalar.dma_start(
            out=wb[2][:].rearrange("p (k c) -> p k c", k=8),
            in_=wqkv[:, 256:384].rearrange("(k p) c -> p k c", p=128))
        # One shared DMA data rail (~1.5us per 512KB), served in enqueue
        # order.  Enqueue only prologue-critical transfers at t=0 (xt, wb2,
        # rope tables); everything else trickles in later via the
        # Pool-paced gpsimd queue so it cannot starve the xt stream.
        for k in (0, 1, 2, 3, 4):
            nc.sync.dma_start(out=xt_t[k][:], in_=xt[ts(k, 128), :])
        for k in (5, 6, 7):
            nc.gpsimd.dma_start(out=xt_t[k][:], in_=xt[ts(k, 128), :])
        # rope tables behind xt on the Pool-paced queue (cannot cut the
        # shared DMA rail ahead of the xt stream)
        nc.gpsimd.dma_start(out=cos_t[:], in_=cosb[:])
        nc.gpsimd.dma_start(out=sin_t[:], in_=sinb[:])
        nc.gpsimd.dma_start(
            out=wv[:].rearrange("p (k c) -> p k c", k=8),
            in_=wqkv[:, 512:768].rearrange("(k p) c -> p k c", p=128))
        nc.gpsimd.dma_start(out=tri_t[:], in_=trid[:])
        nc.gpsimd.dma_start(out=t0_t[:], in_=t0row[:])
        nc.gpsimd.dma_start(out=ident_t[:], in_=ident[:])
        nc.vector.memset(ones1[:], 1.0)

        # xt arrival order on the shared rail
        KORD = [0, 1, 5, 2, 6, 3, 4, 7]

        # ---------------- helpers ----------------------------------------
        def rope_piece(m, src_ap, gcol, copy_eng=None):
            """qkT[m][:, gcol:gcol+512] = src*cos + shuffle(src)*sin.

            The psum piece is evacuated to bf16 first (frees the psum WAR
            fast; GPSIMD cannot touch PSUM, so this is ACT in the prologue
            and DVE elsewhere); the arithmetic runs on DVE in all-bf16
            SBUF (fast 2x/4x modes)."""
            sl = ds(gcol, 512)
            qb = rope_p.tile([128, 512], BF16, tag="qb",
                             name=f"qb{m}_{gcol}")
            if copy_eng is nc.scalar:
                nc.scalar.copy(qb[:], src_ap)
            else:
                nc.vector.tensor_copy(qb[:], src_ap)
            a = rope_p.tile([128, 512], BF16, tag="ra",
                            name=f"ra{m}_{gcol}")
            rs = rope_p.tile([128, 512], BF16, tag="rs",
                             name=f"rs{m}_{gcol}")
            b = rope_p.tile([128, 512], BF16, tag="rb",
                            name=f"rb{m}_{gcol}")
            nc.vector.tensor_mul(a[:], qb[:], cos_t[:, sl])
            nc.vector.stream_shuffle(rs[:], qb[:], _SHUF)
            nc.vector.tensor_mul(b[:], rs[:], sin_t[:, sl])
            nc.vector.tensor_add(qkT[m][:, sl], a[:], b[:])

        # ---------------- prologue: q01, k01, v(T0-3) --------------------
        # q01 into two [128,1024] psum halves, k-loop interleaved
        # dummy matmuls keep the PE busy-period alive from t~0 while the xt
        # DMAs stream in: instruction cost is priced at dispatch with the
        # ramp known then, so a burst released after a stall is priced at
        # the LOW p-state.  The dummy source is a memset tile (no DMA dep).
        dumsrc = consts.tile([128, 512], BF16, tag="dumsrc")
        nc.vector.memset(dumsrc[:], 0.0)
        dum = y_ps.tile([128, 512], F32, tag="y", name="dum")

        def dummy():
            nc.tensor.matmul(dum[:], dumsrc[:, 0:128], dumsrc[:],
                             start=True, stop=True)

        # q01 + k01-lo interleaved in one chunk-chasing loop: q01 fills the
        # two qk-pool tiles (lo first: QK-T0 lands on b0, WARing only the
        # early q-lo ropes); k01-lo accumulates in the av/dn banks
        psq_lo = qk_ps.tile([128, 1024], F32, tag="qk", name="psq_lo")
        psq_hi = qk_ps.tile([128, 1024], F32, tag="qk", name="psq_hi")
        ka = av_ps.tile([128, 512], F32, tag="av", name="k01a")
        kb = dn_ps.tile([128, 512], F32, tag="dn", name="k01b")
        for _ in range(4):
            dummy()
        for ki, k in enumerate(KORD):
            for hi, pp in ((0, psq_lo), (1, psq_hi)):
                for j in range(2):
                    nc.tensor.matmul(
                        pp[:, ts(j, 512)],
                        wb[0][:, ts(k, 128)],
                        xt_t[k][:, ds(1024 * hi + 512 * j, 512)],
                        start=(ki == 0), stop=(ki == 7))
            nc.tensor.matmul(ka[:], wb[2][:, ts(k, 128)],
                             xt_t[k][:, ds(0, 512)],
                             start=(ki == 0), stop=(ki == 7))
            nc.tensor.matmul(kb[:], wb[2][:, ts(k, 128)],
                             xt_t[k][:, ds(512, 512)],
                             start=(ki == 0), stop=(ki == 7))
            if ki < 7:
                for _ in range(2):
                    dummy()

        # rope order: q-lo pieces unblock QK-T0's WAR; k-p0 is its data
        # dep.  ACT (idle before the first exp) does the psum-evac copies.
        rope_piece(0, psq_lo[:, ts(0, 512)], 0, copy_eng=nc.scalar)
        rope_piece(0, psq_lo[:, ts(1, 512)], 512, copy_eng=nc.scalar)
        rope_piece(2, ka[:], 0, copy_eng=nc.scalar)
        rope_piece(0, psq_hi[:, ts(0, 512)], 1024, copy_eng=nc.scalar)
        rope_piece(0, psq_hi[:, ts(1, 512)], 1536, copy_eng=nc.scalar)
        rope_piece(2, kb[:], 512, copy_eng=nc.scalar)

        # v(T0..T3) is emitted as slot items at the start of h0-sh0 (wv
        # lands after the xt stream; attention QK/exp starts first)

        # late weights: Pool generates these descriptors after the rope
        # copies have drained (emission order = Pool SEQ order)
        for m in (1, 3):
            nc.gpsimd.dma_start(
                out=wb[m][:].rearrange("p (k c) -> p k c", k=8),
                in_=wqkv[:, ts(m, 128)].rearrange("(k p) c -> p k c", p=128))
        for p in range(2):
            nc.gpsimd.dma_start(out=wout_t[p][:], in_=wout[ts(p, 128), :])

        # ---------------- interleaved work queues ------------------------
        # v split by head-pair: heads 0/1 during h0-sh0 (needed first),
        # heads 2/3 during the otherwise-idle h1-sh1 window
        vq0 = deque((T0, 0) for T0 in range(0, NT, 2))
        vq1 = deque((T0, 1) for T0 in range(0, NT, 2))
        khq = deque([2, 3])           # k01-hi s-pieces, during h0-sh0

        def kh_item(p):
            yp = y_ps.tile([128, 512], F32, tag="y", name=f"kh{p}")
            for k in range(8):
                nc.tensor.matmul(yp[:], wb[2][:, ts(k, 128)],
                                 xt_t[k][:, ds(512 * p, 512)],
                                 start=(k == 0), stop=(k == 7))
            rope_piece(2, yp[:], 512 * p)

        def v_item(T0, half):
            # v projections (2 T-chunks x 2 heads) share the y-psum bank
            vs = y_ps.tile([128, 512], F32, tag="y", name=f"vs{T0}_{half}")
            for t2 in range(2):
                T = T0 + t2
                for k in range(8):
                    nc.tensor.matmul(vs[:, ds(256 * t2, 128)],
                                     xt_t[k][:, ts(T, 128)],
                                     wv[:, ds(256 * k + 128 * half, 128)],
                                     start=(k == 0), stop=(k == 7))
                nc.vector.tensor_copy(
                    v_all[:, ds(256 * T + 128 * half, 128)],
                    vs[:, ds(256 * t2, 128)])

        # q23/k23 projection pieces through the y-psum bank
        projmms = deque()

        def _mk_proj(m, p):
            yp = y_ps.tile([128, 512], F32, tag="y", name=f"pj{m}_{p}")

            def mk_mm(k):
                def f():
                    nc.tensor.matmul(yp[:], wb[m][:, ts(k, 128)],
                                     xt_t[k][:, ds(512 * p, 512)],
                                     start=(k == 0), stop=(k == 7))
                    if k == 7:
                        rope_piece(m, yp[:], 512 * p)
                return f
            return [mk_mm(k) for k in range(8)]

        projq = deque((m, p) for m in (1, 3) for p in range(4))

        def pump_proj():
            if not projmms and projq:
                projmms.extend(_mk_proj(*projq.popleft()))
            for _ in range(2):
                if projmms:
                    projmms.popleft()()

        # out_proj items
        yq0 = deque((0, e, sc) for sc in range(4) for e in range(8))
        yq1a = deque((1, e, sc) for sc in range(2) for e in range(8))
        yq1b = deque((1, e, sc) for sc in (2, 3) for e in range(8))
        ycnt = [0]

        # y evacuations batch in [128,1024] staging tiles: one DMA per two
        # out_proj tiles (the HWDGE per-transfer overhead is ~625ns)
        ybuf = {}

        def y_item(pair, e, sc, pool=None, tag="y", act_evac=False,
                   by_e=False):
            ycnt[0] += 1
            yp = (pool or y_ps).tile([128, 512], F32, tag=tag,
                                     name=f"y{pair}_{e}_{sc}")
            nc.tensor.matmul(yp[:], wout_t[pair][:, ts(e, 128)],
                             oT2[pair][:, ts(sc, 512)],
                             start=True, stop=True)
            key = (pair, e // 2, sc) if by_e else (pair, e, sc // 2)
            slot = (e % 2) if by_e else (sc % 2)
            if key not in ybuf:
                ybuf[key] = [ysb_p.tile([128, 1024], BF16, tag="ysb",
                                        name=f"ysb{key[0]}_{key[1]}_{key[2]}"
                                        f"{'e' if by_e else 's'}"),
                             0]
            ent = ybuf[key]
            dst = ent[0][:, ds(512 * slot, 512)]
            if act_evac:
                nc.scalar.copy(dst, yp[:])
            else:
                nc.vector.tensor_copy(dst, yp[:])
            ent[1] += 1
            if ent[1] == 2:
                if by_e:
                    nc.sync.dma_start(
                        out=yt2[pair, ds(256 * (e // 2), 256), ts(sc, 512)]
                        .rearrange("(two p) c -> p two c", p=128),
                        in_=ent[0][:].rearrange("p (two c) -> p two c",
                                                two=2))
                else:
                    nc.sync.dma_start(
                        out=yt2[pair, ts(e, 128),
                                ds(1024 * (sc // 2), 1024)],
                        in_=ent[0][:])
                del ybuf[key]

        # ---------------- attention --------------------------------------
        def emit_qk(h, shi, T, ps):
            qq = qkT[h // 2]
            kk = qkT[2 + h // 2]
            r0 = 64 * (h % 2)
            for j in range(2):
                nc.tensor.matmul(
                    ps[:, ts(j, 512)],
                    kk[r0:r0 + 64, ts(T, 128)],
                    qq[r0:r0 + 64, ds(1024 * shi + 512 * j, 512)],
                    start=True, stop=True)

        def emit_bands(shi, T, unn):
            lo = 1024 * shi
            b0 = 128 * T
            if lo <= b0 < lo + 1024:
                c = b0 - lo
                nc.gpsimd.tensor_mul(unn[:, ds(c, 128)], unn[:, ds(c, 128)],
                                     tri_t[:, 0:128])
            b1 = b0 + 128
            if b1 < S and lo <= b1 < lo + 1024:
                c = b1 - lo
                if T % 2 == 0:
                    nc.gpsimd.memset(unn[:, ds(c, 128)], 0.0)
                else:
                    nc.gpsimd.tensor_mul(unn[:, ds(c, 128)],
                                         unn[:, ds(c, 128)],
                                         tri_t[:, 128:256])
            if T == 0:
                nc.vector.tensor_mul(unn[0:4, :], unn[0:4, :],
                                     t0_t[:, ds(lo, 1024)])

        def emit_avdn(h, T, unn, av, dn):
            first, last = (T == 0), (T == NT - 1)
            for c in range(8):
                nc.tensor.matmul(av[:, ts(c, 64)], unn[:, ts(c, 128)],
                                 v_all[:, ds(256 * T + 64 * h, 64)],
                                 start=(first and c == 0),
                                 stop=(last and c == 7))
            for c in range(8):
                nc.tensor.matmul(dn[:, c:c + 1], unn[:, ts(c, 128)],
                                 ones1[:],
                                 start=(first and c == 0),
                                 stop=(last and c == 7))

        def make_norm(h, shi, av, dn):
            """Split norm: DVE part (recip + scalar-muls into osb tiles)
            emitted at the next loop's T1; the PE transposes + oT2 copies
            are pumped as two thunks at T2/T3 so they don't sit between
            consecutive QK matmuls while DVE catches up."""
            p = h // 2
            r0 = 64 * (h % 2)
            osbs = []

            def dve_part():
                rec = rec_p.tile([128, 8], F32, tag="rec",
                                 name=f"rec{h}_{shi}")
                nc.vector.reciprocal(rec[:], dn[:, 0:8])
                for c in range(8):
                    osb = osb_p.tile([128, 64], BF16, tag="osb",
                                     name=f"o{h}_{shi}_{c}")
                    nc.vector.tensor_scalar_mul(
                        osb[:], av[:, ds(64 * c, 64)], rec[:, c:c + 1])
                    osbs.append(osb)

            def mk_quad(q):
                def quad():
                    tr = tr_ps.tile([64, 512], BF16, tag="tr",
                                    name=f"tr{h}_{shi}_{q}")
                    for j in range(4):
                        nc.tensor.transpose(tr[:, ds(128 * j, 128)],
                                            osbs[4 * q + j][:], ident_t[:])
                    nc.vector.tensor_copy(
                        oT2[p][r0:r0 + 64, ds(1024 * shi + 512 * q, 512)],
                        tr[:])
                return quad
            return dve_part, deque([mk_quad(0), mk_quad(1)])

        prev_dve = [None]
        tr_thunks = deque()

        def pump(h, shi, T):
            g = (h * 2 + shi) * 16 + T
            if tr_thunks and T >= 2:
                tr_thunks.popleft()()
            if 16 <= g < 48:
                pump_proj()
            elif g >= 64:
                if yq0 and g >= 66:
                    y_item(*yq0.popleft())
                elif yq1a and g >= 116:
                    y_item(*yq1a.popleft())
                    if yq1a and g >= 122:
                        y_item(*yq1a.popleft())

        def attn(h, shi):
            av = av_ps.tile([128, 512], F32, tag="av", name=f"av{h}_{shi}")
            dn = dn_ps.tile([128, 512], F32, tag="dn", name=f"dn{h}_{shi}")
            deferred = []
            for T in range(NT):
                ps = qk_ps.tile([128, 1024], F32, tag="qk",
                                name=f"qk{h}_{shi}_{T}")
                emit_qk(h, shi, T, ps)
                unn = unn_p.tile([128, 1024], BF16, tag="unn",
                                 name=f"unn{h}_{shi}_{T}")
                nc.scalar.activation(unn[:], ps[:], EXP, scale=0.125)
                if T == 1 and prev_dve[0] is not None:
                    prev_dve[0]()
                    prev_dve[0] = None
                emit_bands(shi, T, unn)
                # v/k-hi projections must precede their consumers in PE
                # order; v leads its AV consumer by the deferral depth
                if h == 0 and shi == 0:
                    if khq:
                        kh_item(khq.popleft())
                    elif vq0:
                        v_item(*vq0.popleft())
                elif h == 1 and shi == 1 and vq1:
                    v_item(*vq1.popleft())
                # AV deferred 3 slots so PE never sits on a cross-engine
                # dependency between consecutive QK matmuls
                deferred.append((T, unn))
                if T >= 6:
                    Td, unnd = deferred.pop(0)
                    emit_avdn(h, Td, unnd, av, dn)
                pump(h, shi, T)
            while deferred:
                Td, unnd = deferred.pop(0)
                emit_avdn(h, Td, unnd, av, dn)
            dve_part, quads = make_norm(h, shi, av, dn)
            prev_dve[0] = dve_part
            tr_thunks.extend(quads)

        for h in range(4):
            for shi in range(2):
                attn(h, shi)

        # ---------------- tail: remaining out_proj -----------------------
        # keep the PE busy-period alive through the norm/transpose chain so
        # the tail out_proj matmuls are priced at the full p-state (the
        # qk psum pool is free once the last exp has drained)
        dumt = qk_ps.tile([128, 1024], F32, tag="qk", name="dumt")

        def tdummy(n):
            for _ in range(n):
                nc.tensor.matmul(dumt[:, 0:512], dumsrc[:, 0:128],
                                 dumsrc[:], start=True, stop=True)

        prev_dve[0]()
        prev_dve[0] = None
        tdummy(12)
        pools = [(y_ps, "y"), (av_ps, "av"), (dn_ps, "dn")]
        # drain: quad0 -> its dependent y items -> quad1 -> the rest, so
        # the first half of the tail is not gated on the last oT2 write.
        # ACT (idle after the last exp) takes half the evacuations.
        cnt = [0]

        def drain(items, by_e=True):
            for pair, e, sc in items:
                pool, tag = pools[cnt[0] % 3]
                # engine per staging PAIR (a shared tile's WAW ordering
                # would serialize mixed engines); ACT takes 2 of 3 pairs
                y_item(pair, e, sc, pool=pool, tag=tag,
                       act_evac=((cnt[0] // 2) % 3 != 0), by_e=by_e)
                cnt[0] += 1

        leftover = list(yq1a)
        yq1a.clear()
        yq1b.clear()
        tr_thunks.popleft()()      # quad0 -> oT2[1] sh1 cols sc2
        drain(leftover, by_e=False)   # completes open in-loop sc-pairs
        drain([(1, e, 2) for e in range(8)])
        tr_thunks.popleft()()      # quad1 -> sc3
        drain([(1, e, 3) for e in range(8)])


# ---------------------------------------------------------------- host side
def _host_consts():
    w_pos = np.arange(S, dtype=np.float64)
    inv_freq = 1.0 / (ROPE_BASE ** (np.arange(0, D, 2, dtype=np.float64) / D))
    freqs = np.outer(w_pos, inv_freq)                    # [S, 32]
    emb = np.concatenate([freqs, freqs], axis=-1)        # [S, 64]
    cosT = np.cos(emb).T                                 # [64, S]
    sinT = np.sin(emb).T
    # shuffled operand is q[d^1]; rot(q)[d] = sign(d) * q[d^1], sign=-1 on
    # even d -> fold into sin
    sgn = np.where(np.arange(D) % 2 == 0, -1.0, 1.0)[:, None]
    sinT = sinT * sgn
    cosb = np.concatenate([cosT, cosT], 0).astype(ml_dtypes.bfloat16)
    sinb = np.concatenate([sinT, sinT], 0).astype(ml_dtypes.bfloat16)

    tl = np.arange(128)[:, None]
    so = np.arange(128)[None, :]
    tri = np.zeros((128, 256), np.float32)
    tri[:, 0:128] = (so < tl)          # TRI_UP: keep s < t
    tri[:, 128:256] = (so > tl)        # TRI_LO: keep s > t
    tri = tri.astype(ml_dtypes.bfloat16)

    t0 = (np.arange(S)[None, :] < np.arange(4)[:, None]) \
        .astype(ml_dtypes.bfloat16)    # keep s < t for global rows
    ident = np.eye(128, dtype=ml_dtypes.bfloat16)
    return cosb, sinb, tri, t0, ident


def core_inputs(x, w_qkv, w_out, c):
    cosb, sinb, tri, t0, ident = _host_consts()
    wq3 = w_qkv.reshape(E, 3, H, D)
    b, g = divmod(c, HL)
    h0 = HL * g
    wq = wq3[:, 0, h0:h0 + HL].reshape(E, 256)
    wk = wq3[:, 1, h0:h0 + HL].reshape(E, 256)
    wv = wq3[:, 2, h0:h0 + HL].reshape(E, 256)
    wqkv_l = np.concatenate(
        [wq[:, 0:128], wq[:, 128:256], wk[:, 0:128], wk[:, 128:256], wv],
        axis=1).astype(ml_dtypes.bfloat16)
    return {
        "xt": np.ascontiguousarray(x[b].T).astype(ml_dtypes.bfloat16),
        "wqkv": wqkv_l,
        "wout": np.ascontiguousarray(
            w_out[h0 * D:h0 * D + 256, :]).astype(ml_dtypes.bfloat16),
        "cosb": cosb, "sinb": sinb, "trid": tri, "t0row": t0,
        "ident": ident,
    }


_NC_CACHE = None
LAST_RUN = None


def kernel(x, w_qkv, w_out):
    global _NC_CACHE, LAST_RUN
    x = np.asarray(x, np.float32)
    w_qkv = np.asarray(w_qkv, np.float32)
    w_out = np.asarray(w_out, np.float32)

    if _NC_CACHE is None:
        _NC_CACHE = build_nc()
    nc = _NC_CACHE

    in_maps = [core_inputs(x, w_qkv, w_out, c) for c in range(N_CORES)]
    res = run_bass_kernel_spmd(nc, in_maps, list(range(N_CORES)))
    LAST_RUN = res
    outs = res.results

    y = np.zeros((B, S, E), np.float32)
    for c in range(N_CORES):
        b = c // HL
        yt = np.asarray(outs[c]["yt2"], dtype=np.float32)
        y[b] += yt[0].T
        y[b] += yt[1].T
    return y


if __name__ == "__main__":
    rng = np.random.default_rng(0)
    x = rng.standard_normal((B, S, E), dtype=np.float32)
    wq = (rng.standard_normal((E, 3 * E), dtype=np.float32) * E ** -0.5)
    wo = (rng.standard_normal((E, E), dtype=np.float32) * E ** -0.5)
    out = kernel(x, wq, wo)
    print(out.shape, out.dtype, np.abs(out).mean())


# revision 77
# speedup vs baseline: 1.0022x; 1.0022x over previous
"""Trainium2 Bass kernel for nn_BlockwiseAttention (sparse_attention).

Full (unsharded) inputs in, full output out.  Internally shards across the
8 NeuronCores as (batch x head-group): core c -> batch c//4, heads
[4*(c%4), 4*(c%4)+4).  Each core computes qkv projection + RoPE + masked
attention + its slice of the output projection; the host sums the per-core
(and per head-pair) partial projections.

v2 design (ACT-bound softmax pipeline):
  - all matmuls in bf16 (cost-model rate 1.0/row, same as fp32r, but
    enables small-free-dim AV matmuls without the fp32r <256 penalty)
  - scores^T tiles [t=128, s=1024] in PSUM; exp on ACT -> unn bf16 SBUF
  - mask applied POST-exp as zeroing: DVE triangle multiplies + Pool
    memset on the in-band blocks (cheap; frees the PE of mask matmuls)
  - AV restructured: out[s,d] (128-partition outputs, 64-wide free) --
    half the PE rows of the [d,s] orientation; denominators via [128,1]
    ones-matmuls into a separate PSUM bank
  - normalize (DVE tensor_scalar), PE-transpose o -> oT [d,s], out_proj
    per head-pair with partial sums finished on the host
  - q23/k23 projections, v projections, and pair-0 out_proj are
    interleaved into the attention T-loops (PE has slack; ACT is the
    bottleneck engine at ~133us busy)
"""

import sys
import numpy as np
from collections import deque

for _p in ("/opt/trn_rl_repo",):
    if _p not in sys.path:
        sys.path.insert(0, _p)

import ml_dtypes

import concourse.bass as bass
import concourse.tile as tile
from concourse import mybir
from concourse.bass import ts, ds
from concourse.bass_utils import run_bass_kernel_spmd

# ---------------------------------------------------------------- constants
B, S, E = 2, 2048, 1024
H, D = 16, 64
HL = 4                     # heads per core
N_CORES = 8
BLOCK, NG, WIN = 256, 4, 128
ROPE_BASE = 10000.0

F32 = mybir.dt.float32
BF16 = mybir.dt.bfloat16
EXP = mybir.ActivationFunctionType.Exp

NT = S // 128              # 16 t-chunks
_SHUF = [i ^ 1 for i in range(32)]


# ---------------------------------------------------------------- program
def build_nc(for_sim: bool = False) -> bass.Bass:
    nc = bass.Bass()

    xt = nc.declare_dram_parameter("xt", [E, S], BF16, isOutput=False)
    wqkv = nc.declare_dram_parameter("wqkv", [E, 768], BF16, isOutput=False)
    wout = nc.declare_dram_parameter("wout", [256, E], BF16, isOutput=False)
    cosb = nc.declare_dram_parameter("cosb", [128, S], BF16, isOutput=False)
    sinb = nc.declare_dram_parameter("sinb", [128, S], BF16, isOutput=False)
    trid = nc.declare_dram_parameter("trid", [128, 256], BF16, isOutput=False)
    t0row = nc.declare_dram_parameter("t0row", [4, S], BF16, isOutput=False)
    ident = nc.declare_dram_parameter("ident", [128, 128], BF16,
                                      isOutput=False)
    yt2 = nc.declare_dram_parameter("yt2", [2, E, S], BF16, isOutput=True)

    with tile.TileContext(nc) as tc:
        _body(nc, tc, xt, wqkv, wout, cosb, sinb, trid, t0row, ident, yt2)
    if not for_sim:
        _split_waits(nc)
    return nc


def _split_waits(nc):
    """walrus's per-instruction sync structs accept few sync waits; peel
    extra waits onto same-engine ENGINE_NOPs inserted right before the
    instruction (the sequencer processes their waits first)."""
    eng = mybir.EngineType
    builders = {eng.PE: nc.tensor, eng.DVE: nc.vector,
                eng.Activation: nc.scalar, eng.Pool: nc.gpsimd,
                eng.SP: nc.sync}
    fn = nc.m.functions[0]

    def mk_nop(builder):
        builder.nop()
        scratch = fn.blocks[-1].instructions
        raw = scratch.pop()
        assert type(raw).__name__ == "InstNoOp", type(raw).__name__
        return raw

    for blk in fn.blocks:
        out = []
        changed = False
        for inst in blk.instructions:
            si = inst.sync_info
            if (si is not None and len(si.on_wait) > 1
                    and inst.engine in builders):
                waits = list(si.on_wait)
                for w in waits[:-1]:
                    nop = mk_nop(builders[inst.engine])
                    nop.engine = inst.engine
                    nop.sync_info = mybir.SyncInfo(on_wait=[w], on_update=[])
                    out.append(nop)
                si.on_wait = [waits[-1]]
                changed = True
            out.append(inst)
        if changed:
            blk.instructions[:] = out
    return nc


def _body(nc, tc, xt, wqkv, wout, cosb, sinb, trid, t0row, ident, yt2):
    from contextlib import ExitStack

    with ExitStack() as ctx:
        # ---------------- SBUF pools -------------------------------------
        consts = ctx.enter_context(tc.tile_pool(name="consts", bufs=1))
        xt_p = ctx.enter_context(tc.tile_pool(name="xt_p", bufs=1))
        w_p = ctx.enter_context(tc.tile_pool(name="w_p", bufs=1))
        qk_sb = ctx.enter_context(tc.tile_pool(name="qk_sb", bufs=1))
        v_sb = ctx.enter_context(tc.tile_pool(name="v_sb", bufs=1))
        rope_p = ctx.enter_context(tc.tile_pool(name="rope_p", bufs=2))
        unn_p = ctx.enter_context(tc.tile_pool(name="unn_p", bufs=8))
        osb_p = ctx.enter_context(tc.tile_pool(name="osb_p", bufs=4))
        rec_p = ctx.enter_context(tc.tile_pool(name="rec_p", bufs=2))
        oT_p = ctx.enter_context(tc.tile_pool(name="oT_p", bufs=1))
        ysb_p = ctx.enter_context(tc.tile_pool(name="ysb_p", bufs=10))

        # ---------------- PSUM pools (16KB budget, creation order) -------
        qk_ps = ctx.enter_context(
            tc.tile_pool(name="qk_ps", bufs=2, space="PSUM"))   # 8KB
        av_ps = ctx.enter_context(
            tc.tile_pool(name="av_ps", bufs=1, space="PSUM"))   # 2KB
        dn_ps = ctx.enter_context(
            tc.tile_pool(name="dn_ps", bufs=1, space="PSUM"))   # 2KB
        y_ps = ctx.enter_context(
            tc.tile_pool(name="y_ps", bufs=1, space="PSUM"))    # 2KB
        tr_ps = ctx.enter_context(
            tc.tile_pool(name="tr_ps", bufs=1, space="PSUM"))   # 1 bank

        # ---------------- SBUF tiles -------------------------------------
        cos_t = consts.tile([128, S], BF16, tag="cos")
        sin_t = consts.tile([128, S], BF16, tag="sin")
        tri_t = consts.tile([128, 256], BF16, tag="tri")
        t0_t = consts.tile([4, S], BF16, tag="t0")
        ident_t = consts.tile([128, 128], BF16, tag="ident")
        wout_t = [consts.tile([128, E], BF16, tag=f"wout{p}",
                              name=f"wout{p}") for p in range(2)]
        ones1 = consts.tile([128, 1], BF16, tag="ones1")

        xt_t = [xt_p.tile([128, S], BF16, tag=f"xt{k}", name=f"xt{k}")
                for k in range(8)]
        # wb[m]: m0=q01 m1=q23 m2=k01 m3=k23 ([128, 8 e-chunks x 128 cols])
        wb = [w_p.tile([128, 1024], BF16, tag=f"wb{m}", name=f"wb{m}")
              for m in range(4)]
        wv = w_p.tile([128, 2048], BF16, tag="wv")
        qkT = [qk_sb.tile([128, S], BF16, tag=f"qkT{m}", name=f"qkT{m}")
               for m in range(4)]
        # v_all block (T, h) at col 256*T + 64*h
        v_all = v_sb.tile([128, NT * 256], BF16, tag="vall")
        oT2 = [oT_p.tile([128, S], BF16, tag=f"oT{p}", name=f"oT{p}")
               for p in range(2)]

        # ---------------- DMA queues -------------------------------------
        # HWDGE(sync) takes the bulk transfers: SWDGE descriptor generation
        # occupies the Pool ENGINE (~2us per big transfer), which we need
        # for compute copies.  gpsimd only loads the small weight blocks.
        nc.gpsimd.dma_start(
            out=wb[0][:].rearrange("p (k c) -> p k c", k=8),
            in_=wqkv[:, 0:128].rearrange("(k p) c -> p k c", p=128))
        nc.scalar.dma_start(
            out=wb[2][:].rearrange("p (k c) -> p k c", k=8),
            in_=wqkv[:, 256:384].rearrange("(k p) c -> p k c", p=128))
        # One shared DMA data rail (~1.5us per 512KB), served in enqueue
        # order.  Enqueue only prologue-critical transfers at t=0 (xt, wb2,
        # rope tables); everything else trickles in later via the
        # Pool-paced gpsimd queue so it cannot starve the xt stream.
        for k in (0, 1, 2, 3, 4):
            nc.sync.dma_start(out=xt_t[k][:], in_=xt[ts(k, 128), :])
        for k in (5, 6, 7):
            nc.gpsimd.dma_start(out=xt_t[k][:], in_=xt[ts(k, 128), :])
        # rope tables behind xt on the Pool-paced queue (cannot cut the
        # shared DMA rail ahead of the xt stream)
        nc.gpsimd.dma_start(out=cos_t[:], in_=cosb[:])
        nc.gpsimd.dma_start(out=sin_t[:], in_=sinb[:])
        nc.gpsimd.dma_start(
            out=wv[:].rearrange("p (k c) -> p k c", k=8),
            in_=wqkv[:, 512:768].rearrange("(k p) c -> p k c", p=128))
        nc.gpsimd.dma_start(out=tri_t[:], in_=trid[:])
        nc.gpsimd.dma_start(out=t0_t[:], in_=t0row[:])
        nc.gpsimd.dma_start(out=ident_t[:], in_=ident[:])
        nc.vector.memset(ones1[:], 1.0)

        # xt arrival order on the shared rail
        KORD = [0, 1, 5, 2, 6, 3, 4, 7]

        # ---------------- helpers ----------------------------------------
        def rope_piece(m, src_ap, gcol, copy_eng=None):
            """qkT[m][:, gcol:gcol+512] = src*cos + shuffle(src)*sin.

            The psum piece is evacuated to bf16 first (frees the psum WAR
            fast; GPSIMD cannot touch PSUM, so this is ACT in the prologue
            and DVE elsewhere); the arithmetic runs on DVE in all-bf16
            SBUF (fast 2x/4x modes)."""
            sl = ds(gcol, 512)
            qb = rope_p.tile([128, 512], BF16, tag="qb",
                             name=f"qb{m}_{gcol}")
            if copy_eng is nc.scalar:
                nc.scalar.copy(qb[:], src_ap)
            else:
                nc.vector.tensor_copy(qb[:], src_ap)
            a = rope_p.tile([128, 512], BF16, tag="ra",
                            name=f"ra{m}_{gcol}")
            rs = rope_p.tile([128, 512], BF16, tag="rs",
                             name=f"rs{m}_{gcol}")
            b = rope_p.tile([128, 512], BF16, tag="rb",
                            name=f"rb{m}_{gcol}")
            nc.vector.tensor_mul(a[:], qb[:], cos_t[:, sl])
            nc.vector.stream_shuffle(rs[:], qb[:], _SHUF)
            nc.vector.tensor_mul(b[:], rs[:], sin_t[:, sl])
            nc.vector.tensor_add(qkT[m][:, sl], a[:], b[:])

        # ---------------- prologue: q01, k01, v(T0-3) --------------------
        # q01 into two [128,1024] psum halves, k-loop interleaved
        # dummy matmuls keep the PE busy-period alive from t~0 while the xt
        # DMAs stream in: instruction cost is priced at dispatch with the
        # ramp known then, so a burst released after a stall is priced at
        # the LOW p-state.  The dummy source is a memset tile (no DMA dep).
        dumsrc = consts.tile([128, 512], BF16, tag="dumsrc")
        nc.vector.memset(dumsrc[:], 0.0)
        dum = y_ps.tile([128, 512], F32, tag="y", name="dum")

        def dummy():
            nc.tensor.matmul(dum[:], dumsrc[:, 0:128], dumsrc[:],
                             start=True, stop=True)

        # q01 + k01-lo interleaved in one chunk-chasing loop: q01 fills the
        # two qk-pool tiles (lo first: QK-T0 lands on b0, WARing only the
        # early q-lo ropes); k01-lo accumulates in the av/dn banks
        psq_lo = qk_ps.tile([128, 1024], F32, tag="qk", name="psq_lo")
        psq_hi = qk_ps.tile([128, 1024], F32, tag="qk", name="psq_hi")
        ka = av_ps.tile([128, 512], F32, tag="av", name="k01a")
        kb = dn_ps.tile([128, 512], F32, tag="dn", name="k01b")
        for _ in range(4):
            dummy()
        for ki, k in enumerate(KORD):
            for hi, pp in ((0, psq_lo), (1, psq_hi)):
                for j in range(2):
                    nc.tensor.matmul(
                        pp[:, ts(j, 512)],
                        wb[0][:, ts(k, 128)],
                        xt_t[k][:, ds(1024 * hi + 512 * j, 512)],
                        start=(ki == 0), stop=(ki == 7))
            nc.tensor.matmul(ka[:], wb[2][:, ts(k, 128)],
                             xt_t[k][:, ds(0, 512)],
                             start=(ki == 0), stop=(ki == 7))
            nc.tensor.matmul(kb[:], wb[2][:, ts(k, 128)],
                             xt_t[k][:, ds(512, 512)],
                             start=(ki == 0), stop=(ki == 7))
            if ki < 7:
                for _ in range(2):
                    dummy()

        # rope order: q-lo pieces unblock QK-T0's WAR; k-p0 is its data
        # dep.  ACT (idle before the first exp) does the psum-evac copies.
        rope_piece(0, psq_lo[:, ts(0, 512)], 0, copy_eng=nc.scalar)
        rope_piece(0, psq_lo[:, ts(1, 512)], 512, copy_eng=nc.scalar)
        rope_piece(2, ka[:], 0, copy_eng=nc.scalar)
        rope_piece(0, psq_hi[:, ts(0, 512)], 1024, copy_eng=nc.scalar)
        rope_piece(0, psq_hi[:, ts(1, 512)], 1536, copy_eng=nc.scalar)
        rope_piece(2, kb[:], 512, copy_eng=nc.scalar)

        # v(T0..T3) is emitted as slot items at the start of h0-sh0 (wv
        # lands after the xt stream; attention QK/exp starts first)

        # late weights: Pool generates these descriptors after the rope
        # copies have drained (emission order = Pool SEQ order)
        for m in (1, 3):
            nc.gpsimd.dma_start(
                out=wb[m][:].rearrange("p (k c) -> p k c", k=8),
                in_=wqkv[:, ts(m, 128)].rearrange("(k p) c -> p k c", p=128))
        for p in range(2):
            nc.gpsimd.dma_start(out=wout_t[p][:], in_=wout[ts(p, 128), :])

        # ---------------- interleaved work queues ------------------------
        # v split by head-pair: heads 0/1 during h0-sh0 (needed first),
        # heads 2/3 during the otherwise-idle h1-sh1 window
        vq0 = deque((T0, 0) for T0 in range(0, NT, 2))
        vq1 = deque((T0, 1) for T0 in range(0, NT, 2))
        khq = deque([2, 3])           # k01-hi s-pieces, during h0-sh0

        def kh_item(p):
            yp = y_ps.tile([128, 512], F32, tag="y", name=f"kh{p}")
            for k in range(8):
                nc.tensor.matmul(yp[:], wb[2][:, ts(k, 128)],
                                 xt_t[k][:, ds(512 * p, 512)],
                                 start=(k == 0), stop=(k == 7))
            rope_piece(2, yp[:], 512 * p)

        def v_item(T0, half):
            # v projections (2 T-chunks x 2 heads) share the y-psum bank
            vs = y_ps.tile([128, 512], F32, tag="y", name=f"vs{T0}_{half}")
            for t2 in range(2):
                T = T0 + t2
                for k in range(8):
                    nc.tensor.matmul(vs[:, ds(256 * t2, 128)],
                                     xt_t[k][:, ts(T, 128)],
                                     wv[:, ds(256 * k + 128 * half, 128)],
                                     start=(k == 0), stop=(k == 7))
                nc.vector.tensor_copy(
                    v_all[:, ds(256 * T + 128 * half, 128)],
                    vs[:, ds(256 * t2, 128)])

        # q23/k23 projection pieces through the y-psum bank
        projmms = deque()

        def _mk_proj(m, p):
            yp = y_ps.tile([128, 512], F32, tag="y", name=f"pj{m}_{p}")

            def mk_mm(k):
                def f():
                    nc.tensor.matmul(yp[:], wb[m][:, ts(k, 128)],
                                     xt_t[k][:, ds(512 * p, 512)],
                                     start=(k == 0), stop=(k == 7))
                    if k == 7:
                        rope_piece(m, yp[:], 512 * p)
                return f
            return [mk_mm(k) for k in range(8)]

        projq = deque((m, p) for m in (1, 3) for p in range(4))

        def pump_proj():
            if not projmms and projq:
                projmms.extend(_mk_proj(*projq.popleft()))
            for _ in range(2):
                if projmms:
                    projmms.popleft()()

        # out_proj items
        yq0 = deque((0, e, sc) for sc in range(4) for e in range(8))
        yq1a = deque((1, e, sc) for sc in range(2) for e in range(8))
        yq1b = deque((1, e, sc) for sc in (2, 3) for e in range(8))
        ycnt = [0]

        # y evacuations batch in [128,1024] staging tiles: one DMA per two
        # out_proj tiles (the HWDGE per-transfer overhead is ~625ns)
        ybuf = {}

        def y_item(pair, e, sc, pool=None, tag="y", act_evac=False,
                   by_e=False):
            ycnt[0] += 1
            yp = (pool or y_ps).tile([128, 512], F32, tag=tag,
                                     name=f"y{pair}_{e}_{sc}")
            nc.tensor.matmul(yp[:], wout_t[pair][:, ts(e, 128)],
                             oT2[pair][:, ts(sc, 512)],
                             start=True, stop=True)
            key = (pair, e // 2, sc) if by_e else (pair, e, sc // 2)
            slot = (e % 2) if by_e else (sc % 2)
            if key not in ybuf:
                ybuf[key] = [ysb_p.tile([128, 1024], BF16, tag="ysb",
                                        name=f"ysb{key[0]}_{key[1]}_{key[2]}"
                                        f"{'e' if by_e else 's'}"),
                             0]
            ent = ybuf[key]
            dst = ent[0][:, ds(512 * slot, 512)]
            if act_evac:
                nc.scalar.copy(dst, yp[:])
            else:
                nc.vector.tensor_copy(dst, yp[:])
            ent[1] += 1
            if ent[1] == 2:
                if by_e:
                    nc.sync.dma_start(
                        out=yt2[pair, ds(256 * (e // 2), 256), ts(sc, 512)]
                        .rearrange("(two p) c -> p two c", p=128),
                        in_=ent[0][:].rearrange("p (two c) -> p two c",
                                                two=2))
                else:
                    nc.sync.dma_start(
                        out=yt2[pair, ts(e, 128),
                                ds(1024 * (sc // 2), 1024)],
                        in_=ent[0][:])
                del ybuf[key]

        # ---------------- attention --------------------------------------
        def emit_qk(h, shi, T, ps):
            qq = qkT[h // 2]
            kk = qkT[2 + h // 2]
            r0 = 64 * (h % 2)
            for j in range(2):
                nc.tensor.matmul(
                    ps[:, ts(j, 512)],
                    kk[r0:r0 + 64, ts(T, 128)],
                    qq[r0:r0 + 64, ds(1024 * shi + 512 * j, 512)],
                    start=True, stop=True)

        def emit_bands(shi, T, unn):
            lo = 1024 * shi
            b0 = 128 * T
            if lo <= b0 < lo + 1024:
                c = b0 - lo
                nc.gpsimd.tensor_mul(unn[:, ds(c, 128)], unn[:, ds(c, 128)],
                                     tri_t[:, 0:128])
            b1 = b0 + 128
            if b1 < S and lo <= b1 < lo + 1024:
                c = b1 - lo
                if T % 2 == 0:
                    nc.gpsimd.memset(unn[:, ds(c, 128)], 0.0)
                else:
                    nc.gpsimd.tensor_mul(unn[:, ds(c, 128)],
                                         unn[:, ds(c, 128)],
                                         tri_t[:, 128:256])
            if T == 0:
                nc.vector.tensor_mul(unn[0:4, :], unn[0:4, :],
                                     t0_t[:, ds(lo, 1024)])

        def emit_avdn(h, T, unn, av, dn):
            first, last = (T == 0), (T == NT - 1)
            for c in range(8):
                nc.tensor.matmul(av[:, ts(c, 64)], unn[:, ts(c, 128)],
                                 v_all[:, ds(256 * T + 64 * h, 64)],
                                 start=(first and c == 0),
                                 stop=(last and c == 7))
            for c in range(8):
                nc.tensor.matmul(dn[:, c:c + 1], unn[:, ts(c, 128)],
                                 ones1[:],
                                 start=(first and c == 0),
                                 stop=(last and c == 7))

        def make_norm(h, shi, av, dn):
            """Split norm: DVE part (recip + scalar-muls into osb tiles)
            emitted at the next loop's T1; the PE transposes + oT2 copies
            are pumped as two thunks at T2/T3 so they don't sit between
            consecutive QK matmuls while DVE catches up."""
            p = h // 2
            r0 = 64 * (h % 2)
            osbs = []

            def dve_part():
                rec = rec_p.tile([128, 8], F32, tag="rec",
                                 name=f"rec{h}_{shi}")
                nc.vector.reciprocal(rec[:], dn[:, 0:8])
                for c in range(8):
                    osb = osb_p.tile([128, 64], BF16, tag="osb",
                                     name=f"o{h}_{shi}_{c}")
                    nc.vector.tensor_scalar_mul(
                        osb[:], av[:, ds(64 * c, 64)], rec[:, c:c + 1])
                    osbs.append(osb)

            def mk_quad(q):
                def quad():
                    tr = tr_ps.tile([64, 512], BF16, tag="tr",
                                    name=f"tr{h}_{shi}_{q}")
                    for j in range(4):
                        nc.tensor.transpose(tr[:, ds(128 * j, 128)],
                                            osbs[4 * q + j][:], ident_t[:])
                    nc.vector.tensor_copy(
                        oT2[p][r0:r0 + 64, ds(1024 * shi + 512 * q, 512)],
                        tr[:])
                return quad
            return dve_part, deque([mk_quad(0), mk_quad(1)])

        prev_dve = [None]
        tr_thunks = deque()

        def pump(h, shi, T):
            g = (h * 2 + shi) * 16 + T
            if tr_thunks and T >= 2:
                tr_thunks.popleft()()
            if 16 <= g < 48:
                pump_proj()
            elif g >= 64:
                # keep DVE free around loop boundaries (the norm chain
                # must not queue behind y evacuations)
                if yq0 and g >= 66 and 2 <= T <= 14:
                    y_item(*yq0.popleft())
                elif yq1a and g >= 116:
                    y_item(*yq1a.popleft())
                    if yq1a and g >= 122:
                        y_item(*yq1a.popleft())

        def attn(h, shi):
            av = av_ps.tile([128, 512], F32, tag="av", name=f"av{h}_{shi}")
            dn = dn_ps.tile([128, 512], F32, tag="dn", name=f"dn{h}_{shi}")
            deferred = []
            for T in range(NT):
                ps = qk_ps.tile([128, 1024], F32, tag="qk",
                                name=f"qk{h}_{shi}_{T}")
                emit_qk(h, shi, T, ps)
                unn = unn_p.tile([128, 1024], BF16, tag="unn",
                                 name=f"unn{h}_{shi}_{T}")
                nc.scalar.activation(unn[:], ps[:], EXP, scale=0.125)
                if T == 1 and prev_dve[0] is not None:
                    prev_dve[0]()
                    prev_dve[0] = None
                emit_bands(shi, T, unn)
                # v/k-hi projections must precede their consumers in PE
                # order; v leads its AV consumer by the deferral depth
                if h == 0 and shi == 0:
                    if khq:
                        kh_item(khq.popleft())
                    elif vq0:
                        v_item(*vq0.popleft())
                elif h == 1 and shi == 1 and vq1 and T >= 2:
                    v_item(*vq1.popleft())
                # AV deferred 3 slots so PE never sits on a cross-engine
                # dependency between consecutive QK matmuls
                deferred.append((T, unn))
                if T >= 6:
                    Td, unnd = deferred.pop(0)
                    emit_avdn(h, Td, unnd, av, dn)
                pump(h, shi, T)
            while deferred:
                Td, unnd = deferred.pop(0)
                emit_avdn(h, Td, unnd, av, dn)
            dve_part, quads = make_norm(h, shi, av, dn)
            prev_dve[0] = dve_part
            tr_thunks.extend(quads)

        for h in range(4):
            for shi in range(2):
                attn(h, shi)

        # ---------------- tail: remaining out_proj -----------------------
        # keep the PE busy-period alive through the norm/transpose chain so
        # the tail out_proj matmuls are priced at the full p-state (the
        # qk psum pool is free once the last exp has drained)
        dumt = qk_ps.tile([128, 1024], F32, tag="qk", name="dumt")

        def tdummy(n):
            for _ in range(n):
                nc.tensor.matmul(dumt[:, 0:512], dumsrc[:, 0:128],
                                 dumsrc[:], start=True, stop=True)

        prev_dve[0]()
        prev_dve[0] = None
        tdummy(12)
        pools = [(y_ps, "y"), (av_ps, "av"), (dn_ps, "dn")]
        # drain: quad0 -> its dependent y items -> quad1 -> the rest, so
        # the first half of the tail is not gated on the last oT2 write.
        # ACT (idle after the last exp) takes half the evacuations.
        cnt = [0]

        def drain(items, by_e=True):
            for pair, e, sc in items:
                pool, tag = pools[cnt[0] % 3]
                # engine per staging PAIR (a shared tile's WAW ordering
                # would serialize mixed engines); ACT takes 2 of 3 pairs
                y_item(pair, e, sc, pool=pool, tag=tag,
                       act_evac=((cnt[0] // 2) % 3 != 0), by_e=by_e)
                cnt[0] += 1

        leftover = list(yq1a)
        yq1a.clear()
        yq1b.clear()
        tr_thunks.popleft()()      # quad0 -> oT2[1] sh1 cols sc2
        drain(leftover, by_e=False)   # completes open in-loop sc-pairs
        drain([(1, e, 2) for e in range(8)])
        tr_thunks.popleft()()      # quad1 -> sc3
        drain([(1, e, 3) for e in range(8)])


# ---------------------------------------------------------------- host side
def _host_consts():
    w_pos = np.arange(S, dtype=np.float64)
    inv_freq = 1.0 / (ROPE_BASE ** (np.arange(0, D, 2, dtype=np.float64) / D))
    freqs = np.outer(w_pos, inv_freq)                    # [S, 32]
    emb = np.concatenate([freqs, freqs], axis=-1)        # [S, 64]
    cosT = np.cos(emb).T                                 # [64, S]
    sinT = np.sin(emb).T
    # shuffled operand is q[d^1]; rot(q)[d] = sign(d) * q[d^1], sign=-1 on
    # even d -> fold into sin
    sgn = np.where(np.arange(D) % 2 == 0, -1.0, 1.0)[:, None]
    sinT = sinT * sgn
    cosb = np.concatenate([cosT, cosT], 0).astype(ml_dtypes.bfloat16)
    sinb = np.concatenate([sinT, sinT], 0).astype(ml_dtypes.bfloat16)

    tl = np.arange(128)[:, None]
    so = np.arange(128)[None, :]
    tri = np.zeros((128, 256), np.float32)
    tri[:, 0:128] = (so < tl)          # TRI_UP: keep s < t
    tri[:, 128:256] = (so > tl)        # TRI_LO: keep s > t
    tri = tri.astype(ml_dtypes.bfloat16)

    t0 = (np.arange(S)[None, :] < np.arange(4)[:, None]) \
        .astype(ml_dtypes.bfloat16)    # keep s < t for global rows
    ident = np.eye(128, dtype=ml_dtypes.bfloat16)
    return cosb, sinb, tri, t0, ident


def core_inputs(x, w_qkv, w_out, c):
    cosb, sinb, tri, t0, ident = _host_consts()
    wq3 = w_qkv.reshape(E, 3, H, D)
    b, g = divmod(c, HL)
    h0 = HL * g
    wq = wq3[:, 0, h0:h0 + HL].reshape(E, 256)
    wk = wq3[:, 1, h0:h0 + HL].reshape(E, 256)
    wv = wq3[:, 2, h0:h0 + HL].reshape(E, 256)
    wqkv_l = np.concatenate(
        [wq[:, 0:128], wq[:, 128:256], wk[:, 0:128], wk[:, 128:256], wv],
        axis=1).astype(ml_dtypes.bfloat16)
    return {
        "xt": np.ascontiguousarray(x[b].T).astype(ml_dtypes.bfloat16),
        "wqkv": wqkv_l,
        "wout": np.ascontiguousarray(
            w_out[h0 * D:h0 * D + 256, :]).astype(ml_dtypes.bfloat16),
        "cosb": cosb, "sinb": sinb, "trid": tri, "t0row": t0,
        "ident": ident,
    }


_NC_CACHE = None
LAST_RUN = None


def kernel(x, w_qkv, w_out):
    global _NC_CACHE, LAST_RUN
    x = np.asarray(x, np.float32)
    w_qkv = np.asarray(w_qkv, np.float32)
    w_out = np.asarray(w_out, np.float32)

    if _NC_CACHE is None:
        _NC_CACHE = build_nc()
    nc = _NC_CACHE

    in_maps = [core_inputs(x, w_qkv, w_out, c) for c in range(N_CORES)]
    res = run_bass_kernel_spmd(nc, in_maps, list(range(N_CORES)))
    LAST_RUN = res
    outs = res.results

    y = np.zeros((B, S, E), np.float32)
    for c in range(N_CORES):
        b = c // HL
        yt = np.asarray(outs[c]["yt2"], dtype=np.float32)
        y[b] += yt[0].T
        y[b] += yt[1].T
    return y


if __name__ == "__main__":
    rng = np.random.default_rng(0)
    x = rng.standard_normal((B, S, E), dtype=np.float32)
    wq = (rng.standard_normal((E, 3 * E), dtype=np.float32) * E ** -0.5)
    wo = (rng.standard_normal((E, E), dtype=np.float32) * E ** -0.5)
    out = kernel(x, wq, wo)
    print(out.shape, out.dtype, np.abs(out).mean())
